# revision 14
# baseline (speedup 1.0000x reference)
"""DiffusionUnit kernel for 8 Trainium2 NeuronCores (Bass/Tile).

Reference computation (N=65536 points, C=256 channels, NS=16 neighbors):
    u_pre = u @ W_pre.T + b_pre
    u_n   = u_pre[idx]                       # [N, NS, C] gather
    m     = mean_k relu(u_n - u_pre[:,None]) # diffusion
    h     = m @ W_v.T + b_v
    out   = relu(BN_trainmode(h)) + u

Sharding: points split across 8 cores (8192 rows each); 256x256 weights
replicated; u_pre all-gathered so every core can gather arbitrary neighbor
rows; BN batch stats all-reduced.

Algebraic simplifications (exact in real arithmetic):
  * b_pre cancels in (u_n - u_pre) -> never applied.
  * b_v shifts mean(h) equally -> cancels through BN centering -> never applied.
  * the 1/NS mean is folded into W_v on the host.

Device pipeline per core:
  A) u_preT tiles: PE matmul (host passes u.T shards so lhsT needs no
     on-device transpose), PSUM->SBUF, store to local DRAM table.
  B) AllGather table -> full [65536, C] table in local DRAM.
  C) per 128-point tile: prefill gather tile with -center (broadcast),
     indirect DMA gather with compute_op=add (center subtract happens in
     the DMA), relu + 4-level pairwise tree sum on DVE, PE transpose,
     second matmul, BN stat accumulation via ones-matmul into PSUM.
  D) AllReduce [1,512] stats; per-channel scale/shift broadcast via PE
     outer product; fused affine+relu+residual; store output shard.
"""

import numpy as np

N = 65536
C = 256
NS = 16
NCORES = 8
NSH = N // NCORES          # 8192 rows per core
P = 128
NT = NSH // P              # 64 tiles per core
BN_EPS = 1e-5

_CACHE = {}
_H_TILES = []


def _build_program(n=N, ncores=NCORES):
    import concourse.bass as bass
    import concourse.mybir as mybir
    import concourse.tile as tile
    from concourse import bacc
    from concourse.masks import make_identity

    f32 = mybir.dt.float32
    i32 = mybir.dt.int32
    Alu = mybir.AluOpType

    nsh = n // ncores
    nt = nsh // P
    nc = bacc.Bacc(
        "TRN2",
        target_bir_lowering=False,
        debug=False,
        num_devices=ncores,
    )

    # ------------------------------------------------------------------ I/O
    uT_sh = nc.dram_tensor("uT_sh", [C, nsh], f32, kind="ExternalInput").ap()
    u_sh = nc.dram_tensor("u_sh", [nsh, C], f32, kind="ExternalInput").ap()
    idx_sh = nc.dram_tensor("idx_sh", [nsh, NS], i32, kind="ExternalInput").ap()
    WpreT = nc.dram_tensor("WpreT", [C, C], f32, kind="ExternalInput").ap()
    WvT = nc.dram_tensor("WvT", [C, C], f32, kind="ExternalInput").ap()
    gb = nc.dram_tensor("gb", [2, C], f32, kind="ExternalInput").ap()
    out_sh = nc.dram_tensor("out_sh", [nsh, C], f32, kind="ExternalOutput").ap()

    rg = [list(range(ncores))]

    with tile.TileContext(nc) as tc:
        with (
            tc.tile_pool(name="const", bufs=1) as const,
            tc.tile_pool(name="work", bufs=3) as work,
            tc.tile_pool(name="hkeep", bufs=nt) as hkeep,
            tc.tile_pool(name="gath", bufs=3) as gath,
            tc.tile_pool(name="psum", bufs=2, space="PSUM") as psum,
            tc.tile_pool(name="psum1", bufs=1, space="PSUM") as psum1,
            tc.tile_pool(name="dram", bufs=1, space="DRAM") as dram,
        ):
            # ------------------------------------------------- constants
            wpre0 = const.tile([P, C], f32)
            wpre1 = const.tile([P, C], f32)
            wv0 = const.tile([P, C], f32)
            wv1 = const.tile([P, C], f32)
            nc.sync.dma_start(out=wpre0[:], in_=WpreT[0:P, :])
            nc.sync.dma_start(out=wpre1[:], in_=WpreT[P:C, :])
            nc.sync.dma_start(out=wv0[:], in_=WvT[0:P, :])
            nc.sync.dma_start(out=wv1[:], in_=WvT[P:C, :])
            gamma_sb = const.tile([1, C], f32)
            nc.sync.dma_start(out=gamma_sb[:], in_=gb[0:1, :])
            beta_sb = const.tile([1, C], f32)
            nc.sync.dma_start(out=beta_sb[:], in_=gb[1:2, :])
            ident = const.tile([P, P], f32)
            make_identity(nc, ident[:])
            ones_col = const.tile([P, 1], f32)
            nc.vector.memset(ones_col[:], 1.0)
            ones_row = const.tile([1, P], f32)
            nc.vector.memset(ones_row[:], 1.0)

            # DRAM scratch
            upre_local = dram.tile([nsh, C], f32)
            upre_full = dram.tile([n, C], f32, addr_space="Shared")
            stats_local = dram.tile([1, 2 * C], f32)
            stats_glob = dram.tile([1, 2 * C], f32, addr_space="Shared")

            # ------------------------------------------------- phase A
            for t in range(nt):
                sl = slice(t * P, (t + 1) * P)
                lhs0 = work.tile([P, P], f32, tag="lhs0")
                lhs1 = work.tile([P, P], f32, tag="lhs1")
                nc.sync.dma_start(out=lhs0[:], in_=uT_sh[0:P, sl])
                nc.sync.dma_start(out=lhs1[:], in_=uT_sh[P:C, sl])
                ps_up = psum.tile([P, C], f32, tag="mm")
                nc.tensor.matmul(out=ps_up[:], lhsT=lhs0[:], rhs=wpre0[:],
                                 start=True, stop=False)
                nc.tensor.matmul(out=ps_up[:], lhsT=lhs1[:], rhs=wpre1[:],
                                 start=False, stop=True)
                up_sb = work.tile([P, C], f32, tag="upsb")
                nc.scalar.copy(out=up_sb[:], in_=ps_up[:])
                nc.sync.dma_start(out=upre_local[sl, :], in_=up_sb[:])

            # ------------------------------------------------- phase B
            nc.gpsimd.collective_compute(
                "AllGather",
                mybir.AluOpType.bypass,
                replica_groups=rg,
                ins=[upre_local[:]],
                outs=[upre_full[:]],
            )

            # ------------------------------------------------- phase C
            st_ps_h = psum1.tile([1, C], f32, tag="sth")
            st_ps_q = psum1.tile([1, C], f32, tag="stq")
            for t in range(nt):
                sl = slice(t * P, (t + 1) * P)
                idx_t = work.tile([P, NS], i32, tag="idxt")
                nc.sync.dma_start(out=idx_t[:], in_=idx_sh[sl, :])
                center = work.tile([P, C], f32, tag="center")
                nc.sync.dma_start(out=center[:], in_=upre_local[sl, :])

                g = gath.tile([P, NS, C], f32, tag="g")
                # prefill with negated center, broadcast across NS slots
                nc.vector.tensor_scalar_mul(
                    g[:], center[:, None, :].to_broadcast([P, NS, C]), -1.0
                )
                # gather one row per partition per neighbor slot; the CCE
                # adds onto the prefill -> u_n - center lands directly.
                # (HW vector-indirect DMA semantics: one offset per
                # partition, so one call per neighbor.)
                for k in range(NS):
                    nc.gpsimd.indirect_dma_start(
                        out=g[:, k, :],
                        out_offset=None,
                        in_=upre_full[:],
                        in_offset=bass.IndirectOffsetOnAxis(
                            ap=idx_t[:, k:k + 1], axis=0),
                        compute_op=Alu.add,
                    )
                g2 = g.rearrange("p a b -> p (a b)")
                nc.vector.tensor_scalar_max(g2[:, :], g2[:, :], 0.0)
                # pairwise tree sum over the NS axis
                half = NS * C // 2
                while half >= C:
                    nc.vector.tensor_tensor(
                        out=g2[:, 0:half], in0=g2[:, 0:half],
                        in1=g2[:, half:2 * half], op=Alu.add,
                    )
                    half //= 2
                m_sb = g2[:, 0:C]

                # transpose m -> mT (two 128x128 blocks via PE)
                mt0 = work.tile([P, P], f32, tag="mt0")
                mt1 = work.tile([P, P], f32, tag="mt1")
                ps_t0 = psum.tile([P, P], f32, tag="tp")
                nc.tensor.transpose(out=ps_t0[:], in_=m_sb[:, 0:P], identity=ident[:])
                nc.scalar.copy(out=mt0[:], in_=ps_t0[:])
                ps_t1 = psum.tile([P, P], f32, tag="tp")
                nc.tensor.transpose(out=ps_t1[:], in_=m_sb[:, P:C], identity=ident[:])
                nc.scalar.copy(out=mt1[:], in_=ps_t1[:])

                ps_h = psum.tile([P, C], f32, tag="mm")
                nc.tensor.matmul(out=ps_h[:], lhsT=mt0[:], rhs=wv0[:],
                                 start=True, stop=False)
                nc.tensor.matmul(out=ps_h[:], lhsT=mt1[:], rhs=wv1[:],
                                 start=False, stop=True)
                h_sb = hkeep.tile([P, C], f32, tag="h")
                nc.vector.tensor_copy(out=h_sb[:], in_=ps_h[:])
                hsq = work.tile([P, C], f32, tag="hsq")
                nc.scalar.activation(
                    out=hsq[:], in_=ps_h[:],
                    func=mybir.ActivationFunctionType.Square,
                )
                nc.tensor.matmul(out=st_ps_h[:], lhsT=ones_col[:], rhs=h_sb[:],
                                 start=(t == 0), stop=(t == nt - 1))
                nc.tensor.matmul(out=st_ps_q[:], lhsT=ones_col[:], rhs=hsq[:],
                                 start=(t == 0), stop=(t == nt - 1))
                _H_TILES.append(h_sb)

            # ------------------------------------------------- phase D
            st_sb = work.tile([1, 2 * C], f32, tag="stsb")
            nc.vector.tensor_copy(out=st_sb[0:1, 0:C], in_=st_ps_h[:])
            nc.vector.tensor_copy(out=st_sb[0:1, C:2 * C], in_=st_ps_q[:])
            nc.sync.dma_start(out=stats_local[:], in_=st_sb[:])
            nc.gpsimd.collective_compute(
                "AllReduce",
                Alu.add,
                replica_groups=rg,
                ins=[stats_local[:]],
                outs=[stats_glob[:]],
            )
            ssb = work.tile([1, 2 * C], f32, tag="ssb")
            nc.sync.dma_start(out=ssb[:], in_=stats_glob[:])

            mu = work.tile([1, C], f32, tag="mu")
            nc.vector.tensor_scalar_mul(mu[:], ssb[0:1, 0:C], 1.0 / n)
            ex2 = work.tile([1, C], f32, tag="ex2")
            nc.vector.tensor_scalar_mul(ex2[:], ssb[0:1, C:2 * C], 1.0 / n)
            var = work.tile([1, C], f32, tag="var")
            nc.vector.tensor_tensor(out=var[:], in0=mu[:], in1=mu[:], op=Alu.mult)
            nc.vector.tensor_tensor(out=var[:], in0=ex2[:], in1=var[:],
                                    op=Alu.subtract)
            # rstd = 1/sqrt(var + eps): reciprocal (DVE, exact-ish) then ACT
            # sqrt, then one Newton rsqrt step to polish sqrt's loose ULPs.
            veps = work.tile([1, C], f32, tag="veps")
            nc.vector.tensor_scalar_add(veps[:], var[:], BN_EPS)
            rcp = work.tile([1, C], f32, tag="rcp")
            nc.vector.reciprocal(out=rcp[:], in_=veps[:])
            rstd = work.tile([1, C], f32, tag="rstd")
            nc.scalar.activation(
                out=rstd[:], in_=rcp[:],
                func=mybir.ActivationFunctionType.Sqrt,
            )
            nt1 = work.tile([1, C], f32, tag="nt1")
            nc.vector.tensor_tensor(out=nt1[:], in0=rstd[:], in1=rstd[:],
                                    op=Alu.mult)
            nc.vector.tensor_tensor(out=nt1[:], in0=nt1[:], in1=veps[:],
                                    op=Alu.mult)
            nc.vector.tensor_scalar(nt1[:], nt1[:], -0.5, 1.5, Alu.mult, Alu.add)
            nc.vector.tensor_tensor(out=rstd[:], in0=rstd[:], in1=nt1[:],
                                    op=Alu.mult)
            sc = work.tile([1, C], f32, tag="sc")
            nc.vector.tensor_tensor(out=sc[:], in0=gamma_sb[:], in1=rstd[:],
                                    op=Alu.mult)
            b2 = work.tile([1, C], f32, tag="b2")
            nc.vector.tensor_tensor(out=b2[:], in0=mu[:], in1=sc[:], op=Alu.mult)
            nc.vector.tensor_tensor(out=b2[:], in0=beta_sb[:], in1=b2[:],
                                    op=Alu.subtract)

            # broadcast [1,C] -> [128,C] via K=1 outer product with ones
            ps_s = psum.tile([P, C], f32, tag="tp")
            nc.tensor.matmul(out=ps_s[:], lhsT=ones_row[:], rhs=sc[:],
                             start=True, stop=True)
            s_tile = const.tile([P, C], f32)
            nc.scalar.copy(out=s_tile[:], in_=ps_s[:])
            ps_b = psum.tile([P, C], f32, tag="tp")
            nc.tensor.matmul(out=ps_b[:], lhsT=ones_row[:], rhs=b2[:],
                             start=True, stop=True)
            b2_tile = const.tile([P, C], f32)
            nc.scalar.copy(out=b2_tile[:], in_=ps_b[:])

            for t in range(nt):
                sl = slice(t * P, (t + 1) * P)
                h_sb = _H_TILES[t]
                u_t = work.tile([P, C], f32, tag="ut")
                nc.sync.dma_start(out=u_t[:], in_=u_sh[sl, :])
                o_t = work.tile([P, C], f32, tag="ot")
                nc.vector.tensor_tensor(out=o_t[:], in0=h_sb[:], in1=s_tile[:],
                                        op=Alu.mult)
                nc.vector.tensor_tensor(out=o_t[:], in0=o_t[:], in1=b2_tile[:],
                                        op=Alu.add)
                nc.vector.tensor_scalar_max(o_t[:], o_t[:], 0.0)
                nc.vector.tensor_tensor(out=o_t[:], in0=o_t[:], in1=u_t[:],
                                        op=Alu.add)
                nc.sync.dma_start(out=out_sh[sl, :], in_=o_t[:])

    nc.compile()
    return nc


def _get_program():
    if "nc" not in _CACHE:
        _H_TILES.clear()
        _CACHE["nc"] = _build_program()
    return _CACHE["nc"]


def kernel(p=None, u=None, o=None, idx=None, W_pre=None, b_pre=None,
           W_v=None, b_v=None, gamma=None, beta=None, **_unused):
    from concourse import bass_utils

    u = np.ascontiguousarray(np.asarray(u, dtype=np.float32))
    idx = np.ascontiguousarray(np.asarray(idx, dtype=np.int32))
    W_pre = np.asarray(W_pre, dtype=np.float32)
    W_v = np.asarray(W_v, dtype=np.float32)
    gamma = np.asarray(gamma, dtype=np.float32)
    beta = np.asarray(beta, dtype=np.float32)

    WpreT = np.ascontiguousarray(W_pre.T)
    WvT = np.ascontiguousarray(W_v.T) / np.float32(NS)
    gb = np.ascontiguousarray(np.stack([gamma, beta]).astype(np.float32))
    uT = np.ascontiguousarray(u.T)

    in_maps = []
    for r in range(NCORES):
        rows = slice(r * NSH, (r + 1) * NSH)
        in_maps.append({
            "uT_sh": np.ascontiguousarray(uT[:, rows]),
            "u_sh": np.ascontiguousarray(u[rows, :]),
            "idx_sh": np.ascontiguousarray(idx[rows, :]),
            "WpreT": WpreT,
            "WvT": WvT,
            "gb": gb,
        })

    import os

    nc = _get_program()
    trace = bool(int(os.environ.get("KERNEL_TRACE", "0")))
    res = bass_utils.run_bass_kernel_spmd(
        nc, in_maps, core_ids=list(range(NCORES)), trace=trace
    )
    _CACHE["last_results"] = res
    out = np.concatenate([res.results[r]["out_sh"] for r in range(NCORES)], axis=0)
    return out.astype(np.float32)


if __name__ == "__main__":
    rng = np.random.default_rng(0)
    u = rng.standard_normal((N, C), dtype=np.float32)
    idx = rng.integers(0, N, size=(N, NS)).astype(np.int32)
    s = 1.0 / np.sqrt(C)
    W_pre = rng.uniform(-s, s, size=(C, C)).astype(np.float32)
    b_pre = rng.uniform(-s, s, size=(C,)).astype(np.float32)
    W_v = rng.uniform(-s, s, size=(C, C)).astype(np.float32)
    b_v = rng.uniform(-s, s, size=(C,)).astype(np.float32)
    gamma = np.ones(C, np.float32)
    beta = np.zeros(C, np.float32)
    out = kernel(u=u, idx=idx, W_pre=W_pre, b_pre=b_pre, W_v=W_v, b_v=b_v,
                 gamma=gamma, beta=beta)
    print(out.shape, out.dtype)
